# revision 25
# baseline (speedup 1.0000x reference)
"""Trainium2 Bass kernel for attribute visual attention.

Computes, for each batch b:
    q      = v @ W_alpha                  # [i, f]   (host-precomputed)
    scores = q @ vf[b]                    # [i, r]
    atten  = softmax(scores, axis=r)
    out[b] = atten @ vf[b].T              # [i, f]

Sharding: data-parallel over batch b across 8 NeuronCores (8 batches per
core). The query projection q is computed on the host (tiny: 0.2 GFLOP)
and shipped pre-transposed, which removes the weights DMA + q matmul
phase from the device critical path.

Numerics / engine strategy:
- scores matmul in fp16 (full accuracy; batch-paired rhs, N=392).
- attend matmul in fp8e4 DoubleRow perf mode (2 r-chunks contracted per
  instruction at double rate). Accuracy is preserved with a hi/lo
  split-fp8 scheme: atten = ah + al and vfT = vh + vl (each fp8), and
  out = ah*vh + ah*vl + al*vh (the al*vl term is ~1e-3 relative and
  dropped). Verified numerically: l2 rel err 1.9e-3 vs the 2e-2 gate.
- The run is DMA-bound in steady state (all transfers serialize on the
  global DMA-engine pool): per-wave traffic = vf(f16) + vfT(fp8 hi+lo)
  + out(f16). Loads for waves >= 2 ride SWDGE (gpsimd); outputs and
  startup loads ride SP/HWDGE. Output DMAs are issued per 2-f-tile
  chunk so the store stream drains continuously.
- Software pipeline: wave h's scores/softmax/transposes overlap wave
  h-1's attend; the last i-tile's transposes are emitted after the
  attend so the PE never stalls on the softmax chain.
- esT (transposed atten, fp8) keeps r rows 196..255 zero via one-time
  startup memsets of both pool rotations; the vfT tiles only hold
  196 real rows so the DoubleRow zero-padding contributes nothing.
- PE warm-up runs on an on-chip zeroed tile; ldweights feed the PE
  clock-ramp monitor across DMA-bound wave boundaries.
"""

import contextlib
import numpy as np
from contextlib import ExitStack

import ml_dtypes
import concourse.bass as bass
import concourse.tile as tile
import concourse.bass_utils as bass_utils
from concourse import bacc, mybir

# Problem shapes (hardcoded per contest contract).
B, F, R, I, V = 64, 2048, 196, 312, 300
NCORES = 8
BL = B // NCORES          # 8 batches per core
NPAIR = BL // 2           # 4 batch-pairs per core
FT = F // 128             # 16 f-tiles
I_TILES = ((0, 128), (128, 128), (256, 56))
KR_TILES = ((0, 98), (98, 98))  # r=196 in two equal DoubleRow halves, no pad
IP = 320                  # padded atten-T free stride (4B-aligned)
IH = 156                  # attend output N-half (2*IH = I)

F16 = mybir.dt.float16
F32 = mybir.dt.float32
F8 = mybir.dt.float8e4
DR = mybir.MatmulPerfMode.DoubleRow
NP_F8 = mybir.dt.np(F8)

WARMUP = 130              # PE clock-ramp matmuls; sized to end as vf0 lands

_CACHE = {}


def _build_body(nc, tc, ctx, qt, vf, vft8, ident, out, reps):
    qtp = ctx.enter_context(tc.tile_pool(name="qt", bufs=1))
    ident_t = qtp.tile([128, 128], F16, tag="ident", name="ident")
    qt_t = qtp.tile([128, FT, I], F16, tag="qt", name="qt")
    # qt gates all scores work: first in the sync queue
    nc.sync.dma_start(qt_t[:], qt[:, :, :])

    # PE warm-up on an on-chip zeroed tile: the clock ramp (0.65 -> 2.4 GHz
    # over ~3us continuous) completes while the startup DMAs stream.
    wz = qtp.tile([128, 128], F16, tag="wz", name="wz")
    with tc.high_priority():
        nc.gpsimd.memset(wz[:], 0.0)
    wu_w = wz[:]
    # dummy Exp pulls the 1.3us LoadActFuncSet off the first softmax's
    # critical path into the idle startup window
    actwarm = qtp.tile([1, 2], F32, tag="actwarm", name="actwarm")
    nc.scalar.activation(actwarm[:], wz[0:1, 0:2],
                         mybir.ActivationFunctionType.Exp)
    with tc.tile_pool(name="wupsum", bufs=1, space=bass.MemorySpace.PSUM) as wup:
        wu = wup.tile([128, 128], F32, tag="wu", name="wu")
        for w in range(WARMUP):
            nc.tensor.matmul(wu[:], wu_w, wu_w,
                             start=(w == 0), stop=(w == WARMUP - 1))

    spsum = ctx.enter_context(
        tc.tile_pool(name="spsum", bufs=2, space=bass.MemorySpace.PSUM))
    vfp = ctx.enter_context(tc.tile_pool(name="vf", bufs=3))
    vftp = ctx.enter_context(tc.tile_pool(name="vft", bufs=3))
    esp = ctx.enter_context(tc.tile_pool(name="es", bufs=6))
    attp = ctx.enter_context(tc.tile_pool(name="atT", bufs=2))
    outp = ctx.enter_context(tc.tile_pool(name="out", bufs=2))
    stat = ctx.enter_context(tc.tile_pool(name="stat", bufs=8))
    opsum = ctx.enter_context(
        tc.tile_pool(name="opsum", bufs=4, space=bass.MemorySpace.PSUM))
    tpsum = ctx.enter_context(
        tc.tile_pool(name="tpsum", bufs=1, space=bass.MemorySpace.PSUM))

    # waves 0-1 load via SP/HWDGE in deadline order: qt, vf0, ident, vf1,
    # vft0, vft1; later waves ride SWDGE, paced by the 3-deep rotation.
    early_vf, early_vft = [], []
    for half in range(min(2, NPAIR * reps)):
        vf_t = vfp.tile([128, FT, 2 * R], F16, tag="vf", name="vf")
        nch = 8 if half == 0 else 4
        w = FT // nch
        for c in range(nch):
            nc.sync.dma_start(vf_t[:, w * c:w * (c + 1), :],
                              vf[half, :, w * c:w * (c + 1), :])
        early_vf.append(vf_t)
        if half == 0:
            nc.sync.dma_start(ident_t[:], ident[:])
    for half in range(min(2, NPAIR * reps)):
        vft_t = {}
        for j in range(2):
            b = 2 * half + j
            for h in range(2):
                vv = vftp.tile([98, 2, F], F8, tag=f"vft{j}{h}",
                               name=f"vft{j}{h}")
                nc.sync.dma_start(
                    vv[:],
                    vft8[b, h, :, :].rearrange("(two p) f -> p two f", two=2))
                vft_t[(j, h)] = vv
        early_vft.append(vft_t)

    def softmax(mi, sp):
        i0, isz = I_TILES[mi]
        negmax = stat.tile([isz, 2], F32, tag="negmax")
        with tc.high_priority():
            nc.vector.tensor_reduce(negmax[:], sp[:],
                                    axis=mybir.AxisListType.X,
                                    op=mybir.AluOpType.max, negate=True)
        sums = stat.tile([isz, 2], F32, tag="sums")
        rcp = stat.tile([isz, 2], F32, tag="rcp")
        atts = []
        for j in range(2):
            es = esp.tile([128, R], F16, tag="es")
            att = esp.tile([128, R], F16, tag="att")
            with tc.high_priority():
                nc.scalar.activation(es[:isz, 0:R], sp[:, j, :],
                                     mybir.ActivationFunctionType.Exp,
                                     bias=negmax[:, j:j + 1],
                                     scale=1.0,
                                     accum_out=sums[:, j:j + 1])
                nc.vector.reciprocal(rcp[:, j:j + 1], sums[:, j:j + 1])
                nc.vector.tensor_scalar_mul(att[:isz, :], es[:isz, :],
                                            rcp[:, j:j + 1])
            atts.append(att)
        return atts

    def transpose_att(mi, j, att, tp_t):
        # transpose atten (f16) -> attenT[r, i-slice] on the PE; the fp8
        # hi/lo split happens in the PSUM->SBUF copy stage
        i0, isz = I_TILES[mi]
        for kr, (r0, rs) in enumerate(KR_TILES):
            with tc.high_priority():
                nc.tensor.transpose(
                    tp_t[kr][0:rs, j, i0:i0 + isz],
                    att[:isz, r0:r0 + rs],
                    ident_t[0:isz, 0:isz])

    # attend: outT[f, i] += vfT_{hv}.T @ attenT_{ha} over the 3 hi/lo terms,
    # fp8 DoubleRow (both r-chunks per instruction), N split in halves of 156
    TERMS = ((0, 0), (0, 1), (1, 0))        # (atten half, vft half)

    def emit_attend(vft_t, esT_t, half_p, rep_p):
        final = (rep_p == reps - 1 and half_p == NPAIR - 1)
        otf = {}
        for j in range(2):
            otf[j] = outp.tile([128, FT, I], F16, tag=f"otf{j}",
                               name=f"otf{j}")
        # final wave interleaves j so both output streams drain early
        order = ([(j, mf) for j in range(2) for mf in range(FT)]
                 if not final else
                 [(j, mf) for mf in range(FT) for j in range(2)])
        nco = 0
        for j, mf in order:
            b = 2 * half_p + j
            op_ = opsum.tile([128, I], F32, tag="op", name="op")
            for ih in range(2):
                for t, (ha, hv) in enumerate(TERMS):
                    nc.tensor.matmul(
                        op_[:, ih * IH:(ih + 1) * IH],
                        vft_t[(j, hv)][:, :, mf * 128:(mf + 1) * 128],
                        esT_t[:, :, j, ha, ih * IH:(ih + 1) * IH],
                        start=(t == 0), stop=(t == len(TERMS) - 1),
                        perf_mode=DR)
            with tc.high_priority():
                if nco % 2 == 0:
                    nc.scalar.copy(otf[j][:, mf, :], op_[:])
                else:
                    nc.vector.tensor_copy(otf[j][:, mf, :], op_[:])
            nco += 1
            # per-4-f-tile output chunks: 887ns transfer > 625ns HWDGE
            # desc-gen keeps the store stream transfer-limited; the final
            # wave splits desc-gen across HWDGE (j=0) and SWDGE (j=1)
            if mf % 4 == 3:
                c = mf // 4
                eng = nc.gpsimd if (final and j == 1) else nc.sync
                eng.dma_start(out[b, :, 4 * c:4 * (c + 1), :],
                              otf[j][:, 4 * c:4 * (c + 1), :])

    prev = None
    for rep in range(reps):
        for half in range(NPAIR):
            if half > 0:
                # PSUM-free PE activity across DMA-bound wave boundaries:
                # standalone weight loads keep the clock-ramp monitor fed
                for _ in range(4):
                    nc.tensor.ldweights(wu_w)
            early = (rep == 0 and half <= 1)
            if early:
                vf_t = early_vf[half]
                vft_t = early_vft[half]
            else:
                vf_t = vfp.tile([128, FT, 2 * R], F16, tag="vf", name="vf")
                for c in range(2):
                    w = FT // 2
                    nc.gpsimd.dma_start(vf_t[:, w * c:w * (c + 1), :],
                                        vf[half, :, w * c:w * (c + 1), :])
                vft_t = {}
                for j in range(2):
                    b = 2 * half + j
                    for h in range(2):
                        vv = vftp.tile([98, 2, F], F8, tag=f"vft{j}{h}",
                                       name=f"vft{j}{h}")
                        nc.gpsimd.dma_start(
                            vv[:],
                            vft8[b, h, :, :].rearrange("(two p) f -> p two f",
                                                       two=2))
                        vft_t[(j, h)] = vv

            tp_t = [tpsum.tile([rs, 2, IP], F16, tag=f"tp{kr}",
                               name=f"tp{kr}")
                    for kr, (r0, rs) in enumerate(KR_TILES)]
            esT_t = attp.tile([98, 2, 2, 2, IP], F8, tag="esT", name="esT")

            # software pipeline: wave h's scores/softmax hide wave h-1's
            # attend; the last i-tile's transposes are emitted AFTER the
            # attend so the PE never waits on that softmax chain
            last_atts = None
            for mi, (i0, isz) in enumerate(I_TILES):
                sp = spsum.tile([isz, 2, R], F32, tag="sp", name="sp")
                for kf in range(FT):
                    nc.tensor.matmul(
                        sp[:], qt_t[:, kf, i0:i0 + isz],
                        vf_t[:, kf, :].rearrange("p (j r) -> p j r", j=2),
                        start=(kf == 0), stop=(kf == FT - 1))
                atts = softmax(mi, sp)
                if mi < len(I_TILES) - 1:
                    for j in range(2):
                        transpose_att(mi, j, atts[j], tp_t)
                else:
                    last_atts = atts

            def drain_esT():
                # fp8 hi/lo split of attenT in the PSUM drain: hi = fp8(attT)
                # on Act, lo = fp8(attT - hi) on DVE
                with tc.high_priority():
                    for kr, (r0, rs) in enumerate(KR_TILES):
                        nc.scalar.copy(esT_t[0:rs, kr, :, 0, :],
                                       tp_t[kr][0:rs, :, :])
                    for kr, (r0, rs) in enumerate(KR_TILES):
                        nc.vector.tensor_sub(esT_t[0:rs, kr, :, 1, :],
                                             tp_t[kr][0:rs, :, :],
                                             esT_t[0:rs, kr, :, 0, :])

            final_wave = (rep == reps - 1 and half == NPAIR - 1)
            if not final_wave:
                # steady waves: the deferred attend hides the last i-tile's
                # softmax chain, and its copies precede the esT drains
                if prev is not None:
                    emit_attend(*prev)
                for j in range(2):
                    transpose_att(len(I_TILES) - 1, j, last_atts[j], tp_t)
                drain_esT()
            else:
                # final wave: no next scores exist, so drain esT FIRST (on
                # idle copy engines) and let the deferred attend overlap it;
                # the final attend then starts without queueing behind the
                # previous attend's 16-deep copy backlog
                for j in range(2):
                    transpose_att(len(I_TILES) - 1, j, last_atts[j], tp_t)
                drain_esT()
                if prev is not None:
                    emit_attend(*prev)
            prev = (vft_t, esT_t, half, rep)

    emit_attend(*prev)


def _get_program(reps=1):
    key = ("nc", reps)
    if key in _CACHE:
        return _CACHE[key]
    nc = bacc.Bacc("TRN2", target_bir_lowering=False, debug=False,
                   num_devices=NCORES)
    qt_d = nc.dram_tensor("qt", [128, FT, I], F16, kind="ExternalInput")
    vf_d = nc.dram_tensor("vf", [NPAIR, 128, FT, 2 * R], F16,
                          kind="ExternalInput")
    vft8_d = nc.dram_tensor("vft8", [BL, 2, R, F], F8, kind="ExternalInput")
    id_d = nc.dram_tensor("ident", [128, 128], F16, kind="ExternalInput")
    out_d = nc.dram_tensor("out", [BL, 128, FT, I], F16,
                           kind="ExternalOutput")

    with tile.TileContext(nc) as tc, ExitStack() as ctx:
        _build_body(nc, tc, ctx, qt_d.ap(), vf_d.ap(), vft8_d.ap(),
                    id_d.ap(), out_d.ap(), reps)
    nc.compile()
    _CACHE[key] = nc
    return nc


def _prep_inputs(visual_features, v, W_alpha):
    vf = np.asarray(visual_features, dtype=np.float32)
    v = np.asarray(v, dtype=np.float32)
    W = np.asarray(W_alpha, dtype=np.float32)

    # host-side query projection: q = v @ W -> qT[f, i] as [p, t, i] f16
    q = (v.astype(np.float64) @ W.astype(np.float64)).astype(np.float32)
    qt16 = np.ascontiguousarray(
        q.T.reshape(FT, 128, I).transpose(1, 0, 2)).astype(np.float16)

    # [b, f, r] -> [bp, p=128, t=16, j*196+r]: batch-paired, per-partition
    # contiguous DMA layout for the scores matmul
    vf16 = np.ascontiguousarray(
        vf.reshape(B // 2, 2, FT, 128, R).transpose(0, 3, 2, 1, 4)
        .reshape(B // 2, 128, FT, 2 * R)).astype(np.float16)

    # transposed copy for the attend, split hi/lo fp8: [b, h, r, f]
    vft32 = np.ascontiguousarray(vf.transpose(0, 2, 1))       # [b, r, f]
    vh = vft32.astype(NP_F8)
    vl = (vft32 - vh.astype(np.float32)).astype(NP_F8)
    vft8 = np.ascontiguousarray(
        np.stack([vh, vl], axis=1))                           # [b, 2, r, f]

    in_maps = []
    for c in range(NCORES):
        in_maps.append({
            "qt": qt16,
            "ident": np.eye(128, dtype=np.float16),
            "vf": np.ascontiguousarray(vf16[c * NPAIR:(c + 1) * NPAIR]),
            "vft8": np.ascontiguousarray(vft8[c * BL:(c + 1) * BL]),
        })
    return in_maps


def kernel(visual_features, v, W_alpha):
    nc = _get_program()
    in_maps = _prep_inputs(visual_features, v, W_alpha)
    res = None
    for attempt in range(3):
        try:
            res = bass_utils.run_bass_kernel_spmd(
                nc, in_maps, core_ids=list(range(NCORES)))
            break
        except Exception:
            # transient NRT_EXEC_UNIT_UNRECOVERABLE wedges have been seen on
            # this fabric; a re-dispatch typically succeeds
            if attempt == 2:
                raise
    outs = [res.results[c]["out"] for c in range(NCORES)]
    buf = np.concatenate(outs, axis=0)          # [B, p=128, t=16, I]
    full = buf.transpose(0, 3, 2, 1).reshape(B, I, F)   # f = t*128 + p
    return np.ascontiguousarray(full).astype(np.float32)
